# revision 46
# baseline (speedup 1.0000x reference)
"""Parallel transformer block (pre-LN attention + MLP), 8-way sequence-parallel
on Trainium2 via Bass/Tile.

Sharding: B*S=4096 tokens split into 8 shards of 512 (cores 0-3 batch 0,
cores 4-7 batch 1). Every core runs the full per-token math with the full
weights; K/V shards are AllGather'd (fp8) within each 4-core batch group.

Precision strategy: the attention path (QKV GEMM, scores, probs, ctx, w_o)
runs in fp8-e4m3 with DoubleRow matmuls where the contraction is >=256 deep.
This is safe because softmax over 2048 unmasked keys makes attn_out tiny
(~0.03 rms) relative to the residual stream, so fp8's ~4% relative error
contributes ~1e-3 absolute to the output. The MLP (the dominant GEMM cost)
runs in bf16, which is at fp32 accuracy scale for the 2e-2 harness tolerance.
Weights are cast host-side (fp8 QKV/w_o scaled by 32 to stay in e4m3's
normal range; bf16 MLP).

exp(mask) is folded host-side into the V rows and the softmax-denominator
weights, so the on-device exp needs no per-k-tile bias and batches two
k-tiles per ACTIVATE.

Layouts: "tm" token-major [token, feature] for LN/residual; "fm"
feature-major [feature, token] for GEMM operands. V is produced token-major
so the gathered V tiles feed the ctx matmul directly (no PE transposes).
Scores are computed transposed ([k, q]); the softmax k-reduction is a
DoubleRow matmul against exp(mask) weights.
"""

import math

import numpy as np

H = 2048
NH = 16
DH = 128
FF = 8192
B = 2
S = 2048
EPS = 1e-5
SCALE = 1.0 / math.sqrt(DH)
WS = 32.0                        # fp8 weight scale for wq/wk/wv/w_o

P = 128
NCORES = 8
TOK = (B * S) // NCORES          # 512 tokens per core
TT = TOK // P                    # 4 token tiles per core
HC = H // P                      # 16 feature chunks of hidden dim
KT = S // P                      # 16 k-tiles per batch
RANKS = 4                        # cores per batch group

_BUILD_CACHE = {}


def _build(apply_bv, apply_bo, apply_b2, apply_g1):
    import concourse.bacc as bacc
    import concourse.bass as bass
    import concourse.mybir as mybir
    import concourse.tile as tile
    from concourse.masks import make_identity

    F32 = mybir.dt.float32
    BF16 = mybir.dt.bfloat16
    FP8 = mybir.dt.float8e4
    AF = mybir.ActivationFunctionType
    ADD = mybir.AluOpType.add
    MULT = mybir.AluOpType.mult
    SUB = mybir.AluOpType.subtract
    DR = mybir.MatmulPerfMode.DoubleRow

    nc = bacc.Bacc("TRN2", target_bir_lowering=False, debug=False,
                   num_devices=NCORES)

    # ---- I/O ----
    x_in = nc.dram_tensor("x", [TOK, H], F32, kind="ExternalInput")
    xbft = nc.dram_tensor("xbft", [P, HC, TOK], BF16, kind="ExternalInput")
    emask = nc.dram_tensor("emask", [S], F32, kind="ExternalInput")
    emaskloc = nc.dram_tensor("emaskloc", [TOK], F32, kind="ExternalInput")
    ln1_g = nc.dram_tensor("ln1_g", [H], F32, kind="ExternalInput")
    ln1_b = nc.dram_tensor("ln1_b", [H], F32, kind="ExternalInput")
    # fp8 DoubleRow weights, host-prepacked (see kernel() below)
    wq = nc.dram_tensor("wq", [HC, P, 8, 2, P], FP8, kind="ExternalInput")
    wk = nc.dram_tensor("wk", [HC, P, 8, 2, P], FP8, kind="ExternalInput")
    wv = nc.dram_tensor("wv", [4, 2, P, 4, 2, 512], FP8, kind="ExternalInput")
    wo = nc.dram_tensor("wo", [4, 2, P, 4, 2, 512], FP8, kind="ExternalInput")
    bqkv32 = nc.dram_tensor("bqkv32", [3 * H], F32, kind="ExternalInput")
    b_o = nc.dram_tensor("b_o", [H], F32, kind="ExternalInput")
    ln2_g = nc.dram_tensor("ln2_g", [H], F32, kind="ExternalInput")
    ln2_b = nc.dram_tensor("ln2_b", [H], F32, kind="ExternalInput")
    w1 = nc.dram_tensor("w1", [64, P, HC, P], BF16, kind="ExternalInput")
    b1 = nc.dram_tensor("b1", [FF], F32, kind="ExternalInput")
    w2 = nc.dram_tensor("w2", [4, 16, P, 4, 512], BF16, kind="ExternalInput")
    b2 = nc.dram_tensor("b2", [H], F32, kind="ExternalInput")
    out = nc.dram_tensor("out", [TOK, H], F32, kind="ExternalOutput")

    from contextlib import ExitStack
    with tile.TileContext(nc) as tc, ExitStack() as _es:
        consts = _es.enter_context(tc.tile_pool(name="consts", bufs=1))
        resid = _es.enter_context(tc.tile_pool(name="resid", bufs=1))
        acts = _es.enter_context(tc.tile_pool(name="acts", bufs=1))
        lnp = _es.enter_context(tc.tile_pool(name="lnp", bufs=2))
        wstream = _es.enter_context(tc.tile_pool(name="wstream", bufs=3))
        kvp = _es.enter_context(tc.tile_pool(name="kvp", bufs=2))
        expp = _es.enter_context(tc.tile_pool(name="expp", bufs=3))
        drains = _es.enter_context(tc.tile_pool(name="drains", bufs=3))
        small = _es.enter_context(tc.tile_pool(name="small", bufs=2))
        ps_mm = _es.enter_context(tc.tile_pool(name="ps_mm", bufs=2, space="PSUM"))
        ps_acc = _es.enter_context(tc.tile_pool(name="ps_acc", bufs=4, space="PSUM"))
        dram = _es.enter_context(tc.tile_pool(name="dram", bufs=1, space="DRAM"))

        # ---------------- constants ----------------
        identf = consts.tile([P, P], F32)
        make_identity(nc, identf[:])
        ident_bf = consts.tile([P, P], BF16)
        nc.vector.tensor_copy(ident_bf[:], identf[:])
        ones_rf = consts.tile([1, P], F32)
        nc.vector.memset(ones_rf[:], 1.0)
        ones_row_bf = consts.tile([1, P], BF16)
        nc.vector.tensor_copy(ones_row_bf[:], ones_rf[:])
        ones_cf = consts.tile([P, 1], F32)
        nc.vector.memset(ones_cf[:], 1.0)
        ones_col_bf = consts.tile([P, 1], BF16)
        nc.vector.tensor_copy(ones_col_bf[:], ones_cf[:])
        eps_t = consts.tile([P, 1], F32)
        nc.vector.memset(eps_t[:], EPS)
        negtwo = consts.tile([P, 1], F32)
        nc.vector.memset(negtwo[:], -2.0)

        g1_sb = consts.tile([P, HC], F32)
        nc.sync.dma_start(g1_sb[:], ln1_g.rearrange("(o p) -> p o", p=P))
        b1ln_sb = consts.tile([P, HC], F32)
        nc.sync.dma_start(b1ln_sb[:], ln1_b.rearrange("(o p) -> p o", p=P))
        g2_sb = consts.tile([P, HC], F32)
        nc.sync.dma_start(g2_sb[:], ln2_g.rearrange("(o p) -> p o", p=P))
        b2ln_sb = consts.tile([P, HC], F32)
        nc.sync.dma_start(b2ln_sb[:], ln2_b.rearrange("(o p) -> p o", p=P))
        bqkv_sb = consts.tile([P, 48], F32)
        nc.sync.dma_start(bqkv_sb[:], bqkv32.rearrange("(o p) -> p o", p=P))
        b1_sb = consts.tile([P, 64], F32)
        nc.sync.dma_start(b1_sb[:], b1.rearrange("(o p) -> p o", p=P))
        em_f32 = consts.tile([P, KT], F32)
        nc.sync.dma_start(em_f32[:], emask.rearrange("(o p) -> p o", p=P))
        em8 = consts.tile([P, KT, 16], FP8)
        nc.vector.tensor_copy(em8[:, :, 0], em_f32[:])
        eml_sb = consts.tile([P, TT], F32)
        nc.sync.dma_start(eml_sb[:], emaskloc.rearrange("(t p) -> p t", p=P))

        def bcast_row(src_ap, ncols, tag):
            t = consts.tile([P, ncols], F32, tag=tag, name="bc_" + tag)
            ap = bass.AP(tensor=src_ap.tensor, offset=src_ap.offset,
                         ap=[[0, P]] + [list(d) for d in src_ap.ap])
            nc.gpsimd.dma_start(out=t[:], in_=ap)
            return t

        bo_bc = bcast_row(b_o[0:H], H, "bo") if apply_bo else None
        b2_bc = bcast_row(b2[0:H], H, "b2") if apply_b2 else None
        bv_bc = bcast_row(bqkv32[4096:6144], H, "bv") if apply_bv else None

        # ---------------- DRAM scratch (half-major for chunked AllGathers) ---
        k_bounce = dram.tile([2, H // 2, TOK], FP8)   # K shard (fm), head halves
        v_bounce = dram.tile([2, TOK, H // 2], FP8)   # V shard (tm), col halves
        k_allh = [dram.tile([RANKS, H // 2, TOK], FP8, name=f"k_all{i}")
                  for i in range(2)]
        v_allh = [dram.tile([RANKS, TOK, H // 2], FP8, name=f"v_all{i}")
                  for i in range(2)]

        # ---------------- LN1 (feature-major, PE-reduced stats) ----------
        h_fm = acts.tile([P, HC, TOK], FP8, tag="hfm")
        with nc.named_scope("ln1"):
            xT = lnp.tile([P, HC, TOK], BF16, tag="lnstage", bufs=1,
                           name="ln1xT")
            nc.sync.dma_start(xT[:], xbft[:])
            sum_ps = ps_acc.tile([1, TOK], F32, tag="acc", name="ln1sum")
            sq_ps = ps_acc.tile([1, TOK], F32, tag="acc", name="ln1sq")
            for c in range(HC):
                nc.tensor.matmul(sum_ps[:], ones_col_bf[:], xT[:, c, :],
                                 start=(c == 0), stop=(c == HC - 1))
            for c in range(HC):
                xsq = lnp.tile([P, TOK], BF16, tag="xsq")
                nc.vector.tensor_tensor(xsq[:], xT[:, c, :], xT[:, c, :], MULT)
                nc.tensor.matmul(sq_ps[:], ones_col_bf[:], xsq[:],
                                 start=(c == 0), stop=(c == HC - 1))
            sum_bf = small.tile([1, TOK], BF16, tag="ln1s", bufs=1)
            with nc.allow_low_precision(reason="ln1 sum to bf16"):
                nc.vector.tensor_copy(sum_bf[:], sum_ps[:])
            sq_bf = small.tile([1, TOK], BF16, tag="ln1q", bufs=1)
            with nc.allow_low_precision(reason="ln1 sumsq to bf16"):
                nc.vector.tensor_copy(sq_bf[:], sq_ps[:])
            s_bc = ps_acc.tile([P, TOK], F32, tag="acc", name="ln1sbc")
            nc.tensor.matmul(s_bc[:], ones_row_bf[:], sum_bf[:],
                             start=True, stop=True)
            q_bc = ps_acc.tile([P, TOK], F32, tag="acc", name="ln1qbc")
            nc.tensor.matmul(q_bc[:], ones_row_bf[:], sq_bf[:],
                             start=True, stop=True)
            mean = lnp.tile([P, TOK], F32, tag="ln1m", bufs=1)
            nc.vector.tensor_scalar(mean[:], s_bc[:], 1.0 / H, None, MULT)
            m2 = lnp.tile([P, TOK], F32, tag="ln1m2", bufs=1)
            nc.vector.tensor_tensor(m2[:], mean[:], mean[:], MULT)
            var = lnp.tile([P, TOK], F32, tag="ln1v", bufs=1)
            nc.vector.scalar_tensor_tensor(var[:], q_bc[:], 1.0 / H, m2[:],
                                           MULT, SUB)
            stdv = lnp.tile([P, TOK], F32, tag="ln1sd", bufs=1)
            nc.scalar.activation(stdv[:], var[:], AF.Sqrt,
                                 bias=eps_t[:], scale=1.0)
            rstd = lnp.tile([P, TOK], BF16, tag="ln1r", bufs=1)
            with nc.allow_low_precision(reason="ln1 rstd bf16"):
                nc.vector.reciprocal(rstd[:], stdv[:])
            mb = lnp.tile([P, TOK], BF16, tag="ln1mb", bufs=1)
            nc.vector.tensor_tensor(mb[:], mean[:], rstd[:], MULT)
            for c in range(HC):
                tmp = lnp.tile([P, TOK], BF16, tag="ln1t")
                nc.vector.tensor_tensor(tmp[:], xT[:, c, :], rstd[:], MULT)
                if apply_g1:
                    tmp2 = lnp.tile([P, TOK], BF16, tag="ln1t2")
                    nc.vector.tensor_tensor(tmp2[:], tmp[:], mb[:], SUB)
                    nc.vector.tensor_scalar(h_fm[:, c, :], tmp2[:],
                                            g1_sb[:, c:c + 1],
                                            b1ln_sb[:, c:c + 1], MULT, ADD)
                else:
                    nc.vector.tensor_tensor(h_fm[:, c, :], tmp[:], mb[:], SUB)
        x_r = x_in.rearrange("(t p) h -> p t h", p=P)

        def layernorm_to_fm(get_src, g_sb, bln_sb, h_out, scope,
                            get_stats=None):
            # phase A: per-token-tile stats + normalize into a staging buffer;
            # phase B: chunk-major transposes so downstream GEMMs (which
            # contract chunk-by-chunk) start before all transposes finish.
            # The staging buffer tag is shared by LN1/LN2 (bufs=1 ring).
            with nc.named_scope(scope):
                h_stage = lnp.tile([P, TT, H], BF16, tag="lnstage", bufs=1,
                                   name="lnstage" + scope)
                for t in range(TT):
                    xt = get_src(t)
                    if get_stats is None:
                        stats = lnp.tile([P, 4, 6], F32, tag="stats")
                        xg = xt.rearrange("p (g f) -> p g f", f=512)
                        for g in range(4):
                            nc.vector.bn_stats(stats[:, g, :], xg[:, g, :])
                        stats_ap = stats[:]
                    else:
                        stats_ap = get_stats(t)
                    mv = lnp.tile([P, 2], F32, tag="mv")
                    nc.vector.bn_aggr(mv[:], stats_ap)
                    std = lnp.tile([P, 1], F32, tag="std")
                    nc.scalar.activation(std[:], mv[:, 1:2], AF.Sqrt,
                                         bias=eps_t[:], scale=1.0)
                    rstd = lnp.tile([P, 1], F32, tag="rstd")
                    nc.vector.reciprocal(rstd[:], std[:])
                    nc.vector.tensor_scalar(h_stage[:, t, :], xt, mv[:, 0:1],
                                            rstd[:], SUB, MULT)
                for c in range(HC):
                    for t in range(TT):
                        tr_ps = ps_mm.tile([P, P], BF16, tag="mm")
                        nc.tensor.transpose(tr_ps[:],
                                            h_stage[:, t, c * P:(c + 1) * P],
                                            ident_bf[:])
                        nc.vector.tensor_scalar(
                            h_out[:, c, t * P:(t + 1) * P], tr_ps[:],
                            g_sb[:, c:c + 1], bln_sb[:, c:c + 1], MULT, ADD)

        # ------------- QKV GEMMs + chunked AllGathers (K/V halves) -------------
        groups = [list(range(RANKS)), list(range(RANKS, 2 * RANKS))]

        def fm_block(w_dram, m, bias_col, dst_dram_rows=None, q_dst=None):
            wt = wstream.tile([P, 8, 2, P], FP8, tag="w", name=f"wfm_{m}")
            nc.sync.dma_start(wt[:], w_dram[m])
            ps = ps_mm.tile([P, TOK], F32, tag="mm")
            for c2 in range(8):
                nc.tensor.matmul(ps[:], wt[:, c2, :, :],
                                 h_fm[:, 2 * c2:2 * c2 + 2, :],
                                 start=(c2 == 0), stop=(c2 == 7),
                                 perf_mode=DR)
            if q_dst is not None:
                nc.vector.tensor_scalar(q_dst, ps[:], bias_col, None, ADD)
            else:
                ksb = drains.tile([P, TOK], FP8, tag="kvdrain")
                nc.vector.tensor_scalar(ksb[:], ps[:], bias_col, None, ADD)
                nc.sync.dma_start(dst_dram_rows, ksb[:])

        def v_slice(s):
            wth = []
            for whalf in range(2):
                wt = wstream.tile([P, 4, 2, 512], FP8, tag="w",
                                  name=f"wv_{s}_{whalf}")
                nc.sync.dma_start(wt[:], wv[s, whalf])
                wth.append(wt)
            for t in range(TT):
                ps = ps_mm.tile([P, 512], F32, tag="mm")
                for c2 in range(8):
                    nc.tensor.matmul(ps[:],
                                     h_fm[:, 2 * c2:2 * c2 + 2,
                                          t * P:(t + 1) * P],
                                     wth[c2 // 4][:, c2 % 4, :, :],
                                     start=(c2 == 0), stop=(c2 == 7),
                                     perf_mode=DR)
                vsb = drains.tile([P, 512], FP8, tag="kvdrain")
                if apply_bv:
                    vtmp = drains.tile([P, 512], F32, tag="vtmp")
                    nc.vector.tensor_tensor(vtmp[:], ps[:],
                                            bv_bc[:, s * 512:(s + 1) * 512],
                                            ADD)
                    nc.vector.tensor_scalar(vsb[:], vtmp[:],
                                            eml_sb[:, t:t + 1], None, MULT)
                else:
                    nc.vector.tensor_scalar(vsb[:], ps[:],
                                            eml_sb[:, t:t + 1], None, MULT)
                nc.sync.dma_start(
                    v_bounce[s // 2, t * P:(t + 1) * P,
                             (s % 2) * 512:(s % 2 + 1) * 512], vsb[:])

        def ag(name, src, dst):
            with nc.named_scope(name):
                nc.gpsimd.collective_compute(
                    "AllGather", mybir.AluOpType.bypass,
                    ins=[src.opt()], outs=[dst.opt()], replica_groups=groups)

        for half in range(2):
            with nc.named_scope("qkv_k"):
                for mh in range(8):
                    m = half * 8 + mh
                    fm_block(wk, m, bqkv_sb[:, 16 + m:17 + m],
                             dst_dram_rows=k_bounce[half, mh * P:(mh + 1) * P, :])
            ag(f"ag_k{half}", k_bounce[half], k_allh[half])
            with nc.named_scope("qkv_v"):
                for sh in range(2):
                    v_slice(half * 2 + sh)
            ag(f"ag_v{half}", v_bounce[half], v_allh[half])

        # ---------------- Q GEMM (fm), paced into attention ----------------
        q_fm = acts.tile([P, NH, TOK], FP8, tag="qfm")

        def q_block(m):
            fm_block(wq, m, bqkv_sb[:, m:m + 1], q_dst=q_fm[:, m, :])

        with nc.named_scope("qkv_q"):
            for m in range(HC):
                q_block(m)

        # ---------------- attention ----------------
        k_all_v = [k_allh[i][:].rearrange("r (hh d) t -> d r hh t", d=P)
                   for i in range(2)]
        v_all_v = [v_allh[i][:].rearrange("r (tt p) h -> p r tt h", p=P)
                   for i in range(2)]
        ctx_fm = acts.tile([P, NH, TOK], FP8, tag="cfm")

        with nc.named_scope("attn"):
            for h in range(NH):
                hh = h % 8
                k_h = kvp.tile([P, RANKS, TOK], FP8, tag="kh")
                nc.sync.dma_start(k_h[:], k_all_v[h // 8][:, :, hh, :])
                v_h = kvp.tile([P, RANKS, TT, P], FP8, tag="vh")
                nc.sync.dma_start(v_h[:],
                                  v_all_v[h // 8][:, :, :, hh * P:(hh + 1) * P])
                v_h4 = v_h[:].rearrange("p r tt c -> p (r tt) c")
                den_ps = ps_acc.tile([1, TOK], F32, tag="acc", name="den")
                ctx_ps = ps_acc.tile([P, TOK], F32, tag="acc", name="ctx")
                for b in range(8):
                    sps = ps_mm.tile([P, 2, TOK], F32, tag="mm")
                    for j in range(2):
                        kt = 2 * b + j
                        r, tt = kt // TT, kt % TT
                        nc.tensor.matmul(sps[:, j, :],
                                         k_h[:, r, tt * P:(tt + 1) * P],
                                         q_fm[:, h, :], start=True, stop=True)
                    ex = expp.tile([P, 2, TOK], FP8, tag="exp")
                    nc.scalar.activation(ex[:], sps[:], AF.Exp,
                                         bias=negtwo[:], scale=SCALE / (WS * WS))
                    nc.tensor.matmul(den_ps[:], em8[:, 2 * b:2 * b + 2, 0:1],
                                     ex[:], start=(b == 0), stop=(b == 7),
                                     perf_mode=DR)
                    nc.tensor.matmul(ctx_ps[:],
                                     v_h4[:, 2 * b:2 * b + 2, :],
                                     ex[:], start=(b == 0), stop=(b == 7),
                                     perf_mode=DR)
                # den bank frees after a cheap copy; the reciprocal runs
                # 128-wide on the broadcast, off the PE critical path.
                den_bf = small.tile([1, TOK], BF16, tag="denb")
                with nc.allow_low_precision(reason="softmax denom to bf16"):
                    nc.vector.tensor_copy(den_bf[:], den_ps[:])
                bc_ps = ps_acc.tile([P, TOK], F32, tag="acc", name="bc")
                nc.tensor.matmul(bc_ps[:], ones_row_bf[:], den_bf[:],
                                 start=True, stop=True)
                rbc = small.tile([P, TOK], BF16, tag="rbc")
                with nc.allow_low_precision(reason="softmax denom recip bf16"):
                    nc.vector.reciprocal(rbc[:], bc_ps[:])
                nc.vector.tensor_tensor(ctx_fm[:, h, :], ctx_ps[:], rbc[:],
                                        MULT)

        # ------------- w_o GEMM (tm out) + residual + LN2 stats -------------
        x1_sb = resid.tile([P, TT, H], F32, tag="x1")
        ln2_stats = [consts.tile([P, 4, 6], F32, tag=f"st2_{t}",
                                 name=f"ln2_stats_{t}") for t in range(TT)]
        inv_ws2 = 1.0 / (WS * WS)
        with nc.named_scope("wo"):
            for s in range(4):
                wth = []
                for half in range(2):
                    wt = wstream.tile([P, 4, 2, 512], FP8, tag="w",
                                      name=f"wo_{s}_{half}")
                    nc.sync.dma_start(wt[:], wo[s, half])
                    wth.append(wt)
                for t in range(TT):
                    ps = ps_mm.tile([P, 512], F32, tag="mm")
                    for c2 in range(8):
                        nc.tensor.matmul(ps[:],
                                         ctx_fm[:, 2 * c2:2 * c2 + 2,
                                                t * P:(t + 1) * P],
                                         wth[c2 // 4][:, c2 % 4, :, :],
                                         start=(c2 == 0), stop=(c2 == 7),
                                         perf_mode=DR)
                    xsl = drains.tile([P, 512], F32, tag="xres", bufs=2)
                    nc.sync.dma_start(xsl[:],
                                      x_in[t * P:(t + 1) * P,
                                           s * 512:(s + 1) * 512])
                    x1sl = x1_sb[:, t, s * 512:(s + 1) * 512]
                    nc.vector.scalar_tensor_tensor(x1sl, ps[:], inv_ws2,
                                                   xsl[:], MULT, ADD)
                    if apply_bo:
                        nc.vector.tensor_tensor(x1sl, x1sl,
                                                bo_bc[:, s * 512:(s + 1) * 512],
                                                ADD)
                    nc.vector.bn_stats(ln2_stats[t][:, s, :], x1sl)

        # ---------------- LN2 -> h2_fm bf16 ----------------
        h2_fm = acts.tile([P, HC, TOK], BF16, tag="h2fm")
        layernorm_to_fm(lambda t: x1_sb[:, t, :], g2_sb, b2ln_sb, h2_fm,
                        "ln2", get_stats=lambda t: ln2_stats[t][:])

        # ---------------- MLP ----------------
        inter = acts.tile([P, 64, TOK], BF16, tag="inter")
        with nc.named_scope("mlp_w1"):
            for m in range(64):
                wt = wstream.tile([P, HC, P], BF16, tag="w", name=f"w1_{m}")
                nc.sync.dma_start(wt[:], w1[m])
                ps = ps_mm.tile([P, TOK], F32, tag="mm")
                for c in range(HC):
                    nc.tensor.matmul(ps[:], wt[:, c, :], h2_fm[:, c, :],
                                     start=(c == 0), stop=(c == HC - 1))
                nc.scalar.activation(inter[:, m, :], ps[:], AF.Gelu,
                                     bias=b1_sb[:, m:m + 1], scale=1.0)

        with nc.named_scope("mlp_w2"):
            for s in range(4):
                accs = [ps_acc.tile([P, 512], F32, tag="acc",
                                    name=f"w2acc_{s}_{t}") for t in range(TT)]
                for piece in range(16):
                    wt = wstream.tile([P, 4, 512], BF16, tag="w",
                                      name=f"w2_{s}_{piece}")
                    nc.sync.dma_start(wt[:], w2[s, piece])
                    for t in range(TT):
                        for f in range(4):
                            nc.tensor.matmul(
                                accs[t][:],
                                inter[:, piece * 4 + f, t * P:(t + 1) * P],
                                wt[:, f, :],
                                start=(piece == 0 and f == 0),
                                stop=(piece == 15 and f == 3))
                for t in range(TT):
                    osb = drains.tile([P, 512], F32, tag="osb", bufs=2)
                    nc.vector.tensor_tensor(osb[:], accs[t][:],
                                            x1_sb[:, t, s * 512:(s + 1) * 512],
                                            ADD)
                    if apply_b2:
                        nc.vector.tensor_tensor(osb[:], osb[:],
                                                b2_bc[:, s * 512:(s + 1) * 512],
                                                ADD)
                    nc.sync.dma_start(out[t * P:(t + 1) * P,
                                          s * 512:(s + 1) * 512], osb[:])

    nc.finalize()
    return nc


def _get_nc(apply_bv, apply_bo, apply_b2, apply_g1):
    key = (apply_bv, apply_bo, apply_b2, apply_g1)
    if key not in _BUILD_CACHE:
        _BUILD_CACHE[key] = _build(*key)
    return _BUILD_CACHE[key]


def kernel(x, mask, ln1_g, ln1_b, w_qkv, b_qkv, w_o, b_o, ln2_g, ln2_b,
           w1, b1, w2, b2):
    import ml_dtypes
    from concourse.bass_utils import run_bass_kernel_spmd

    FP8NP = ml_dtypes.float8_e4m3
    BF16NP = ml_dtypes.bfloat16

    f32 = lambda a: np.ascontiguousarray(np.asarray(a), dtype=np.float32)
    x = f32(x)
    mask = f32(mask)
    w_qkv = f32(w_qkv)
    w_o = f32(w_o)
    w1h = f32(w1)
    w2h = f32(w2)
    b_qkv = f32(b_qkv)

    def prep_fm_dr(w, scale):
        # [H, 16*128 cols] -> [16 m, 128 p, 8 c2, 2 j, 128] fp8, x scale
        ww = (w * scale).reshape(8, 2, P, HC, P)          # [c2, j, p, m, mc]
        return np.ascontiguousarray(
            ww.transpose(3, 2, 0, 1, 4)).astype(FP8NP)    # [m, p, c2, j, mc]

    def prep_tm_dr(w, scale):
        # [H, 4*512 cols] -> [4 s, 2 half, 128 p, 4 c2, 2 j, 512] fp8
        ww = (w * scale).reshape(2, 4, 2, P, 4, 512)  # [half, c2, j, p, s, n]
        return np.ascontiguousarray(
            ww.transpose(4, 0, 3, 1, 2, 5)).astype(FP8NP)

    weights = {
        "ln1_g": f32(ln1_g), "ln1_b": f32(ln1_b),
        "ln2_g": f32(ln2_g), "ln2_b": f32(ln2_b),
        "wq": prep_fm_dr(w_qkv[:, 0:H], WS),
        "wk": prep_fm_dr(w_qkv[:, H:2 * H], WS),
        "wv": prep_tm_dr(w_qkv[:, 2 * H:3 * H], WS),
        "wo": prep_tm_dr(w_o, WS),
        "bqkv32": np.ascontiguousarray(b_qkv * WS),
        "b_o": f32(b_o),
        # w1 [H, FF] -> [64 m, 128 p, 16 c, 128] bf16
        "w1": np.ascontiguousarray(
            w1h.reshape(HC, P, 64, P).transpose(2, 1, 0, 3)).astype(BF16NP),
        "b1": f32(b1),
        # w2 [FF, H] -> [4 s, 16 piece, 128 p, 4 f, 512] bf16
        "w2": np.ascontiguousarray(
            w2h.reshape(16, 4, P, 4, 512).transpose(3, 0, 2, 1, 4)).astype(BF16NP),
        "b2": f32(b2),
    }
    nc = _get_nc(bool(np.any(b_qkv[2 * H:])),
                 bool(np.any(weights["b_o"])),
                 bool(np.any(weights["b2"])),
                 bool(np.any(weights["ln1_g"] != 1.0)
                      or np.any(weights["ln1_b"])))

    x_flat = x.reshape(B * S, H)
    em_full = np.exp(mask[:, 0, 0, :].astype(np.float32))   # [B, S]
    in_maps = []
    for c in range(NCORES):
        b = c // RANKS
        xs = x_flat[c * TOK:(c + 1) * TOK]
        m = {"x": np.ascontiguousarray(xs),
             "xbft": np.ascontiguousarray(
                 xs.T.reshape(HC, P, TOK).transpose(1, 0, 2)).astype(BF16NP),
             "emask": np.ascontiguousarray(em_full[b]),
             "emaskloc": np.ascontiguousarray(
                 em_full[b, (c % RANKS) * TOK:(c % RANKS + 1) * TOK])}
        m.update(weights)
        in_maps.append(m)

    res = run_bass_kernel_spmd(nc, in_maps, core_ids=list(range(NCORES)))
    out = np.concatenate([res.results[c]["out"] for c in range(NCORES)], axis=0)
    return out.reshape(B, S, H)


# revision 47
# speedup vs baseline: 1.0345x; 1.0345x over previous
"""Parallel transformer block (pre-LN attention + MLP), 8-way sequence-parallel
on Trainium2 via Bass/Tile.

Sharding: B*S=4096 tokens split into 8 shards of 512 (cores 0-3 batch 0,
cores 4-7 batch 1). Every core runs the full per-token math with the full
weights; K/V shards are AllGather'd (fp8) within each 4-core batch group.

Precision strategy: the attention path (QKV GEMM, scores, probs, ctx, w_o)
runs in fp8-e4m3 with DoubleRow matmuls where the contraction is >=256 deep.
This is safe because softmax over 2048 unmasked keys makes attn_out tiny
(~0.03 rms) relative to the residual stream, so fp8's ~4% relative error
contributes ~1e-3 absolute to the output. The MLP (the dominant GEMM cost)
runs in bf16, which is at fp32 accuracy scale for the 2e-2 harness tolerance.
Weights are cast host-side (fp8 QKV/w_o scaled by 32 to stay in e4m3's
normal range; bf16 MLP).

exp(mask) is folded host-side into the V rows and the softmax-denominator
weights, so the on-device exp needs no per-k-tile bias and batches two
k-tiles per ACTIVATE.

Layouts: "tm" token-major [token, feature] for LN/residual; "fm"
feature-major [feature, token] for GEMM operands. V is produced token-major
so the gathered V tiles feed the ctx matmul directly (no PE transposes).
Scores are computed transposed ([k, q]); the softmax k-reduction is a
DoubleRow matmul against exp(mask) weights.
"""

import math

import numpy as np

H = 2048
NH = 16
DH = 128
FF = 8192
B = 2
S = 2048
EPS = 1e-5
SCALE = 1.0 / math.sqrt(DH)
WS = 32.0                        # fp8 weight scale for wq/wk/wv/w_o

P = 128
NCORES = 8
TOK = (B * S) // NCORES          # 512 tokens per core
TT = TOK // P                    # 4 token tiles per core
HC = H // P                      # 16 feature chunks of hidden dim
KT = S // P                      # 16 k-tiles per batch
RANKS = 4                        # cores per batch group

_BUILD_CACHE = {}


def _build(apply_bv, apply_bo, apply_b2):
    import concourse.bacc as bacc
    import concourse.bass as bass
    import concourse.mybir as mybir
    import concourse.tile as tile
    from concourse.masks import make_identity

    F32 = mybir.dt.float32
    BF16 = mybir.dt.bfloat16
    FP8 = mybir.dt.float8e4
    AF = mybir.ActivationFunctionType
    ADD = mybir.AluOpType.add
    MULT = mybir.AluOpType.mult
    SUB = mybir.AluOpType.subtract
    DR = mybir.MatmulPerfMode.DoubleRow

    nc = bacc.Bacc("TRN2", target_bir_lowering=False, debug=False,
                   num_devices=NCORES)

    # ---- I/O ----
    x_in = nc.dram_tensor("x", [TOK, H], F32, kind="ExternalInput")
    emask = nc.dram_tensor("emask", [S], F32, kind="ExternalInput")
    emaskloc = nc.dram_tensor("emaskloc", [TOK], F32, kind="ExternalInput")
    ln1_g = nc.dram_tensor("ln1_g", [H], F32, kind="ExternalInput")
    ln1_b = nc.dram_tensor("ln1_b", [H], F32, kind="ExternalInput")
    # fp8 DoubleRow weights, host-prepacked (see kernel() below)
    wq = nc.dram_tensor("wq", [HC, P, 8, 2, P], FP8, kind="ExternalInput")
    wk = nc.dram_tensor("wk", [HC, P, 8, 2, P], FP8, kind="ExternalInput")
    wv = nc.dram_tensor("wv", [4, 2, P, 4, 2, 512], FP8, kind="ExternalInput")
    wo = nc.dram_tensor("wo", [4, 2, P, 4, 2, 512], FP8, kind="ExternalInput")
    bqkv32 = nc.dram_tensor("bqkv32", [3 * H], F32, kind="ExternalInput")
    b_o = nc.dram_tensor("b_o", [H], F32, kind="ExternalInput")
    ln2_g = nc.dram_tensor("ln2_g", [H], F32, kind="ExternalInput")
    ln2_b = nc.dram_tensor("ln2_b", [H], F32, kind="ExternalInput")
    w1 = nc.dram_tensor("w1", [64, P, HC, P], BF16, kind="ExternalInput")
    b1 = nc.dram_tensor("b1", [FF], F32, kind="ExternalInput")
    w2 = nc.dram_tensor("w2", [4, 16, P, 4, 512], BF16, kind="ExternalInput")
    b2 = nc.dram_tensor("b2", [H], F32, kind="ExternalInput")
    out = nc.dram_tensor("out", [TOK, H], F32, kind="ExternalOutput")

    from contextlib import ExitStack
    with tile.TileContext(nc) as tc, ExitStack() as _es:
        consts = _es.enter_context(tc.tile_pool(name="consts", bufs=1))
        resid = _es.enter_context(tc.tile_pool(name="resid", bufs=1))
        acts = _es.enter_context(tc.tile_pool(name="acts", bufs=1))
        lnp = _es.enter_context(tc.tile_pool(name="lnp", bufs=2))
        wstream = _es.enter_context(tc.tile_pool(name="wstream", bufs=3))
        kvp = _es.enter_context(tc.tile_pool(name="kvp", bufs=2))
        expp = _es.enter_context(tc.tile_pool(name="expp", bufs=3))
        drains = _es.enter_context(tc.tile_pool(name="drains", bufs=3))
        small = _es.enter_context(tc.tile_pool(name="small", bufs=2))
        ps_mm = _es.enter_context(tc.tile_pool(name="ps_mm", bufs=2, space="PSUM"))
        ps_acc = _es.enter_context(tc.tile_pool(name="ps_acc", bufs=4, space="PSUM"))
        dram = _es.enter_context(tc.tile_pool(name="dram", bufs=1, space="DRAM"))

        # ---------------- constants ----------------
        identf = consts.tile([P, P], F32)
        make_identity(nc, identf[:])
        ident_bf = consts.tile([P, P], BF16)
        nc.vector.tensor_copy(ident_bf[:], identf[:])
        ones_rf = consts.tile([1, P], F32)
        nc.vector.memset(ones_rf[:], 1.0)
        ones_row_bf = consts.tile([1, P], BF16)
        nc.vector.tensor_copy(ones_row_bf[:], ones_rf[:])
        eps_t = consts.tile([P, 1], F32)
        nc.vector.memset(eps_t[:], EPS)
        negtwo = consts.tile([P, 1], F32)
        nc.vector.memset(negtwo[:], -2.0)

        g1_sb = consts.tile([P, HC], F32)
        nc.sync.dma_start(g1_sb[:], ln1_g.rearrange("(o p) -> p o", p=P))
        b1ln_sb = consts.tile([P, HC], F32)
        nc.sync.dma_start(b1ln_sb[:], ln1_b.rearrange("(o p) -> p o", p=P))
        g2_sb = consts.tile([P, HC], F32)
        nc.sync.dma_start(g2_sb[:], ln2_g.rearrange("(o p) -> p o", p=P))
        b2ln_sb = consts.tile([P, HC], F32)
        nc.sync.dma_start(b2ln_sb[:], ln2_b.rearrange("(o p) -> p o", p=P))
        bqkv_sb = consts.tile([P, 48], F32)
        nc.sync.dma_start(bqkv_sb[:], bqkv32.rearrange("(o p) -> p o", p=P))
        b1_sb = consts.tile([P, 64], F32)
        nc.sync.dma_start(b1_sb[:], b1.rearrange("(o p) -> p o", p=P))
        em_f32 = consts.tile([P, KT], F32)
        nc.sync.dma_start(em_f32[:], emask.rearrange("(o p) -> p o", p=P))
        em8 = consts.tile([P, KT, 16], FP8)
        nc.vector.tensor_copy(em8[:, :, 0], em_f32[:])
        eml_sb = consts.tile([P, TT], F32)
        nc.sync.dma_start(eml_sb[:], emaskloc.rearrange("(t p) -> p t", p=P))

        def bcast_row(src_ap, ncols, tag):
            t = consts.tile([P, ncols], F32, tag=tag, name="bc_" + tag)
            ap = bass.AP(tensor=src_ap.tensor, offset=src_ap.offset,
                         ap=[[0, P]] + [list(d) for d in src_ap.ap])
            nc.gpsimd.dma_start(out=t[:], in_=ap)
            return t

        bo_bc = bcast_row(b_o[0:H], H, "bo") if apply_bo else None
        b2_bc = bcast_row(b2[0:H], H, "b2") if apply_b2 else None
        bv_bc = bcast_row(bqkv32[4096:6144], H, "bv") if apply_bv else None

        # ---------------- DRAM scratch (half-major for chunked AllGathers) ---
        k_bounce = dram.tile([2, H // 2, TOK], FP8)   # K shard (fm), head halves
        v_bounce = dram.tile([2, TOK, H // 2], FP8)   # V shard (tm), col halves
        k_allh = [dram.tile([RANKS, H // 2, TOK], FP8, name=f"k_all{i}")
                  for i in range(2)]
        v_allh = [dram.tile([RANKS, TOK, H // 2], FP8, name=f"v_all{i}")
                  for i in range(2)]

        # ---------------- LN1 (token-major) -> h_fm fp8 ----------------
        h_fm = acts.tile([P, HC, TOK], FP8, tag="hfm")
        x_r = x_in.rearrange("(t p) h -> p t h", p=P)

        def layernorm_to_fm(get_src, g_sb, bln_sb, h_out, scope,
                            get_stats=None):
            # phase A: per-token-tile stats + normalize into a staging buffer;
            # phase B: chunk-major transposes so downstream GEMMs (which
            # contract chunk-by-chunk) start before all transposes finish.
            # The staging buffer tag is shared by LN1/LN2 (bufs=1 ring).
            with nc.named_scope(scope):
                h_stage = lnp.tile([P, TT, H], BF16, tag="lnstage", bufs=1,
                                   name="lnstage" + scope)
                for t in range(TT):
                    xt = get_src(t)
                    if get_stats is None:
                        stats = lnp.tile([P, 4, 6], F32, tag="stats")
                        xg = xt.rearrange("p (g f) -> p g f", f=512)
                        for g in range(4):
                            nc.vector.bn_stats(stats[:, g, :], xg[:, g, :])
                        stats_ap = stats[:]
                    else:
                        stats_ap = get_stats(t)
                    mv = lnp.tile([P, 2], F32, tag="mv")
                    nc.vector.bn_aggr(mv[:], stats_ap)
                    std = lnp.tile([P, 1], F32, tag="std")
                    nc.scalar.activation(std[:], mv[:, 1:2], AF.Sqrt,
                                         bias=eps_t[:], scale=1.0)
                    rstd = lnp.tile([P, 1], F32, tag="rstd")
                    nc.vector.reciprocal(rstd[:], std[:])
                    nc.vector.tensor_scalar(h_stage[:, t, :], xt, mv[:, 0:1],
                                            rstd[:], SUB, MULT)
                for c in range(HC):
                    for t in range(TT):
                        tr_ps = ps_mm.tile([P, P], BF16, tag="mm")
                        nc.tensor.transpose(tr_ps[:],
                                            h_stage[:, t, c * P:(c + 1) * P],
                                            ident_bf[:])
                        nc.vector.tensor_scalar(
                            h_out[:, c, t * P:(t + 1) * P], tr_ps[:],
                            g_sb[:, c:c + 1], bln_sb[:, c:c + 1], MULT, ADD)

        def ln1_src(t):
            xt = lnp.tile([P, H], F32, tag="lnx")
            nc.sync.dma_start(xt[:], x_r[:, t, :])
            return xt[:]

        layernorm_to_fm(ln1_src, g1_sb, b1ln_sb, h_fm, "ln1")

        # ------------- QKV GEMMs + chunked AllGathers (K/V halves) -------------
        groups = [list(range(RANKS)), list(range(RANKS, 2 * RANKS))]

        def fm_block(w_dram, m, bias_col, dst_dram_rows=None, q_dst=None):
            wt = wstream.tile([P, 8, 2, P], FP8, tag="w", name=f"wfm_{m}")
            nc.sync.dma_start(wt[:], w_dram[m])
            ps = ps_mm.tile([P, TOK], F32, tag="mm")
            for c2 in range(8):
                nc.tensor.matmul(ps[:], wt[:, c2, :, :],
                                 h_fm[:, 2 * c2:2 * c2 + 2, :],
                                 start=(c2 == 0), stop=(c2 == 7),
                                 perf_mode=DR)
            if q_dst is not None:
                nc.vector.tensor_scalar(q_dst, ps[:], bias_col, None, ADD)
            else:
                ksb = drains.tile([P, TOK], FP8, tag="kvdrain")
                nc.vector.tensor_scalar(ksb[:], ps[:], bias_col, None, ADD)
                nc.sync.dma_start(dst_dram_rows, ksb[:])

        def v_slice(s):
            wth = []
            for whalf in range(2):
                wt = wstream.tile([P, 4, 2, 512], FP8, tag="w",
                                  name=f"wv_{s}_{whalf}")
                nc.sync.dma_start(wt[:], wv[s, whalf])
                wth.append(wt)
            for t in range(TT):
                ps = ps_mm.tile([P, 512], F32, tag="mm")
                for c2 in range(8):
                    nc.tensor.matmul(ps[:],
                                     h_fm[:, 2 * c2:2 * c2 + 2,
                                          t * P:(t + 1) * P],
                                     wth[c2 // 4][:, c2 % 4, :, :],
                                     start=(c2 == 0), stop=(c2 == 7),
                                     perf_mode=DR)
                vsb = drains.tile([P, 512], FP8, tag="kvdrain")
                if apply_bv:
                    vtmp = drains.tile([P, 512], F32, tag="vtmp")
                    nc.vector.tensor_tensor(vtmp[:], ps[:],
                                            bv_bc[:, s * 512:(s + 1) * 512],
                                            ADD)
                    nc.vector.tensor_scalar(vsb[:], vtmp[:],
                                            eml_sb[:, t:t + 1], None, MULT)
                else:
                    nc.vector.tensor_scalar(vsb[:], ps[:],
                                            eml_sb[:, t:t + 1], None, MULT)
                nc.sync.dma_start(
                    v_bounce[s // 2, t * P:(t + 1) * P,
                             (s % 2) * 512:(s % 2 + 1) * 512], vsb[:])

        def ag(name, src, dst):
            with nc.named_scope(name):
                nc.gpsimd.collective_compute(
                    "AllGather", mybir.AluOpType.bypass,
                    ins=[src.opt()], outs=[dst.opt()], replica_groups=groups)

        for half in range(2):
            with nc.named_scope("qkv_k"):
                for mh in range(8):
                    m = half * 8 + mh
                    fm_block(wk, m, bqkv_sb[:, 16 + m:17 + m],
                             dst_dram_rows=k_bounce[half, mh * P:(mh + 1) * P, :])
            ag(f"ag_k{half}", k_bounce[half], k_allh[half])
            with nc.named_scope("qkv_v"):
                for sh in range(2):
                    v_slice(half * 2 + sh)
            ag(f"ag_v{half}", v_bounce[half], v_allh[half])

        # ---------------- Q GEMM (fm), paced into attention ----------------
        q_fm = acts.tile([P, NH, TOK], FP8, tag="qfm")

        def q_block(m):
            fm_block(wq, m, bqkv_sb[:, m:m + 1], q_dst=q_fm[:, m, :])

        with nc.named_scope("qkv_q"):
            for m in range(HC):
                q_block(m)

        # ---------------- attention ----------------
        k_all_v = [k_allh[i][:].rearrange("r (hh d) t -> d r hh t", d=P)
                   for i in range(2)]
        v_all_v = [v_allh[i][:].rearrange("r (tt p) h -> p r tt h", p=P)
                   for i in range(2)]
        ctx_fm = acts.tile([P, NH, TOK], FP8, tag="cfm")

        with nc.named_scope("attn"):
            for h in range(NH):
                hh = h % 8
                k_h = kvp.tile([P, RANKS, TOK], FP8, tag="kh")
                nc.sync.dma_start(k_h[:], k_all_v[h // 8][:, :, hh, :])
                v_h = kvp.tile([P, RANKS, TT, P], FP8, tag="vh")
                nc.sync.dma_start(v_h[:],
                                  v_all_v[h // 8][:, :, :, hh * P:(hh + 1) * P])
                v_h4 = v_h[:].rearrange("p r tt c -> p (r tt) c")
                den_ps = ps_acc.tile([1, TOK], F32, tag="acc", name="den")
                ctx_ps = ps_acc.tile([P, TOK], F32, tag="acc", name="ctx")
                for b in range(8):
                    sps = ps_mm.tile([P, 2, TOK], F32, tag="mm")
                    for j in range(2):
                        kt = 2 * b + j
                        r, tt = kt // TT, kt % TT
                        nc.tensor.matmul(sps[:, j, :],
                                         k_h[:, r, tt * P:(tt + 1) * P],
                                         q_fm[:, h, :], start=True, stop=True)
                    ex = expp.tile([P, 2, TOK], FP8, tag="exp")
                    nc.scalar.activation(ex[:], sps[:], AF.Exp,
                                         bias=negtwo[:], scale=SCALE / (WS * WS))
                    nc.tensor.matmul(den_ps[:], em8[:, 2 * b:2 * b + 2, 0:1],
                                     ex[:], start=(b == 0), stop=(b == 7),
                                     perf_mode=DR)
                    nc.tensor.matmul(ctx_ps[:],
                                     v_h4[:, 2 * b:2 * b + 2, :],
                                     ex[:], start=(b == 0), stop=(b == 7),
                                     perf_mode=DR)
                # den bank frees after a cheap copy; the reciprocal runs
                # 128-wide on the broadcast, off the PE critical path.
                den_bf = small.tile([1, TOK], BF16, tag="denb")
                with nc.allow_low_precision(reason="softmax denom to bf16"):
                    nc.vector.tensor_copy(den_bf[:], den_ps[:])
                bc_ps = ps_acc.tile([P, TOK], F32, tag="acc", name="bc")
                nc.tensor.matmul(bc_ps[:], ones_row_bf[:], den_bf[:],
                                 start=True, stop=True)
                rbc = small.tile([P, TOK], BF16, tag="rbc")
                with nc.allow_low_precision(reason="softmax denom recip bf16"):
                    nc.vector.reciprocal(rbc[:], bc_ps[:])
                nc.vector.tensor_tensor(ctx_fm[:, h, :], ctx_ps[:], rbc[:],
                                        MULT)

        # ------------- w_o GEMM (tm out) + residual + LN2 stats -------------
        x1_sb = resid.tile([P, TT, H], F32, tag="x1")
        ln2_stats = [consts.tile([P, 4, 6], F32, tag=f"st2_{t}",
                                 name=f"ln2_stats_{t}") for t in range(TT)]
        inv_ws2 = 1.0 / (WS * WS)
        with nc.named_scope("wo"):
            for s in range(4):
                wth = []
                for half in range(2):
                    wt = wstream.tile([P, 4, 2, 512], FP8, tag="w",
                                      name=f"wo_{s}_{half}")
                    nc.sync.dma_start(wt[:], wo[s, half])
                    wth.append(wt)
                for t in range(TT):
                    ps = ps_mm.tile([P, 512], F32, tag="mm")
                    for c2 in range(8):
                        nc.tensor.matmul(ps[:],
                                         ctx_fm[:, 2 * c2:2 * c2 + 2,
                                                t * P:(t + 1) * P],
                                         wth[c2 // 4][:, c2 % 4, :, :],
                                         start=(c2 == 0), stop=(c2 == 7),
                                         perf_mode=DR)
                    xsl = drains.tile([P, 512], F32, tag="xres", bufs=2)
                    nc.sync.dma_start(xsl[:],
                                      x_in[t * P:(t + 1) * P,
                                           s * 512:(s + 1) * 512])
                    x1sl = x1_sb[:, t, s * 512:(s + 1) * 512]
                    nc.vector.scalar_tensor_tensor(x1sl, ps[:], inv_ws2,
                                                   xsl[:], MULT, ADD)
                    if apply_bo:
                        nc.vector.tensor_tensor(x1sl, x1sl,
                                                bo_bc[:, s * 512:(s + 1) * 512],
                                                ADD)
                    nc.vector.bn_stats(ln2_stats[t][:, s, :], x1sl)

        # ---------------- LN2 -> h2_fm bf16 ----------------
        h2_fm = acts.tile([P, HC, TOK], BF16, tag="h2fm")
        layernorm_to_fm(lambda t: x1_sb[:, t, :], g2_sb, b2ln_sb, h2_fm,
                        "ln2", get_stats=lambda t: ln2_stats[t][:])

        # ---------------- MLP ----------------
        inter = acts.tile([P, 64, TOK], BF16, tag="inter")
        with nc.named_scope("mlp_w1"):
            for m in range(64):
                wt = wstream.tile([P, HC, P], BF16, tag="w", name=f"w1_{m}")
                nc.sync.dma_start(wt[:], w1[m])
                ps = ps_mm.tile([P, TOK], F32, tag="mm")
                for c in range(HC):
                    nc.tensor.matmul(ps[:], wt[:, c, :], h2_fm[:, c, :],
                                     start=(c == 0), stop=(c == HC - 1))
                nc.scalar.activation(inter[:, m, :], ps[:], AF.Gelu,
                                     bias=b1_sb[:, m:m + 1], scale=1.0)

        with nc.named_scope("mlp_w2"):
            for s in range(4):
                accs = [ps_acc.tile([P, 512], F32, tag="acc",
                                    name=f"w2acc_{s}_{t}") for t in range(TT)]
                for piece in range(16):
                    wt = wstream.tile([P, 4, 512], BF16, tag="w",
                                      name=f"w2_{s}_{piece}")
                    nc.sync.dma_start(wt[:], w2[s, piece])
                    for t in range(TT):
                        for f in range(4):
                            nc.tensor.matmul(
                                accs[t][:],
                                inter[:, piece * 4 + f, t * P:(t + 1) * P],
                                wt[:, f, :],
                                start=(piece == 0 and f == 0),
                                stop=(piece == 15 and f == 3))
                for t in range(TT):
                    osb = drains.tile([P, 512], F32, tag="osb", bufs=2)
                    nc.vector.tensor_tensor(osb[:], accs[t][:],
                                            x1_sb[:, t, s * 512:(s + 1) * 512],
                                            ADD)
                    if apply_b2:
                        nc.vector.tensor_tensor(osb[:], osb[:],
                                                b2_bc[:, s * 512:(s + 1) * 512],
                                                ADD)
                    nc.sync.dma_start(out[t * P:(t + 1) * P,
                                          s * 512:(s + 1) * 512], osb[:])

    nc.finalize()
    return nc


def _get_nc(apply_bv, apply_bo, apply_b2):
    key = (apply_bv, apply_bo, apply_b2)
    if key not in _BUILD_CACHE:
        _BUILD_CACHE[key] = _build(*key)
    return _BUILD_CACHE[key]


def kernel(x, mask, ln1_g, ln1_b, w_qkv, b_qkv, w_o, b_o, ln2_g, ln2_b,
           w1, b1, w2, b2):
    import ml_dtypes
    from concourse.bass_utils import run_bass_kernel_spmd

    FP8NP = ml_dtypes.float8_e4m3
    BF16NP = ml_dtypes.bfloat16

    f32 = lambda a: np.ascontiguousarray(np.asarray(a), dtype=np.float32)
    x = f32(x)
    mask = f32(mask)
    w_qkv = f32(w_qkv)
    w_o = f32(w_o)
    w1h = f32(w1)
    w2h = f32(w2)
    b_qkv = f32(b_qkv)

    def prep_fm_dr(w, scale):
        # [H, 16*128 cols] -> [16 m, 128 p, 8 c2, 2 j, 128] fp8, x scale
        ww = (w * scale).reshape(8, 2, P, HC, P)          # [c2, j, p, m, mc]
        return np.ascontiguousarray(
            ww.transpose(3, 2, 0, 1, 4)).astype(FP8NP)    # [m, p, c2, j, mc]

    def prep_tm_dr(w, scale):
        # [H, 4*512 cols] -> [4 s, 2 half, 128 p, 4 c2, 2 j, 512] fp8
        ww = (w * scale).reshape(2, 4, 2, P, 4, 512)  # [half, c2, j, p, s, n]
        return np.ascontiguousarray(
            ww.transpose(4, 0, 3, 1, 2, 5)).astype(FP8NP)

    weights = {
        "ln1_g": f32(ln1_g), "ln1_b": f32(ln1_b),
        "ln2_g": f32(ln2_g), "ln2_b": f32(ln2_b),
        "wq": prep_fm_dr(w_qkv[:, 0:H], WS),
        "wk": prep_fm_dr(w_qkv[:, H:2 * H], WS),
        "wv": prep_tm_dr(w_qkv[:, 2 * H:3 * H], WS),
        "wo": prep_tm_dr(w_o, WS),
        "bqkv32": np.ascontiguousarray(b_qkv * WS),
        "b_o": f32(b_o),
        # w1 [H, FF] -> [64 m, 128 p, 16 c, 128] bf16
        "w1": np.ascontiguousarray(
            w1h.reshape(HC, P, 64, P).transpose(2, 1, 0, 3)).astype(BF16NP),
        "b1": f32(b1),
        # w2 [FF, H] -> [4 s, 16 piece, 128 p, 4 f, 512] bf16
        "w2": np.ascontiguousarray(
            w2h.reshape(16, 4, P, 4, 512).transpose(3, 0, 2, 1, 4)).astype(BF16NP),
        "b2": f32(b2),
    }
    nc = _get_nc(bool(np.any(b_qkv[2 * H:])),
                 bool(np.any(weights["b_o"])),
                 bool(np.any(weights["b2"])))

    x_flat = x.reshape(B * S, H)
    em_full = np.exp(mask[:, 0, 0, :].astype(np.float32))   # [B, S]
    in_maps = []
    for c in range(NCORES):
        b = c // RANKS
        m = {"x": np.ascontiguousarray(x_flat[c * TOK:(c + 1) * TOK]),
             "emask": np.ascontiguousarray(em_full[b]),
             "emaskloc": np.ascontiguousarray(
                 em_full[b, (c % RANKS) * TOK:(c % RANKS + 1) * TOK])}
        m.update(weights)
        in_maps.append(m)

    res = run_bass_kernel_spmd(nc, in_maps, core_ids=list(range(NCORES)))
    out = np.concatenate([res.results[c]["out"] for c in range(NCORES)], axis=0)
    return out.reshape(B, S, H)
